# revision 7
# baseline (speedup 1.0000x reference)
"""Trainium2 Bass kernel v3 — full-K=128 zero-padded attention matmuls.

Multi-head attention (B=2, N=4096, D=768, H=12, d_head=64) on 8 NeuronCores.
Data-parallel over batch (4 cores per element), tensor-parallel over heads
(3 heads per core). Host sums the 4 partial outputs per batch element.

v3 rationale: half-array matmuls (K=64 contractions / M=65 stationaries)
leave the PE HAM activity monitor unconvinced and the clock gate parks at
1.2 GHz. Padding every attention matmul to the full 128-partition
contraction (zeros in the unused half) costs no extra cycles per
instruction (cycles = moving columns) and keeps the array fully driven, so
the 2.4 GHz clock holds. Heads are processed sequentially instead of
row/col-packed.

Layout (per head h, fp16):
  qTz[h]/kTz[h] [128, N]: head data in rows 0..63 (h0,h2) or 64..127 (h1,
      matching where the fused projection writes), zeros elsewhere.
  v128[h] [128, NKC, 128]: cols 0..63 = V, col 64 = ones (softmax
      denominator), cols 65..127 = zeros.
  A_z[h] [128, N]: normalized attention output rows 0..63, h2 row 64 = ones
      (bias row), zeros elsewhere.
  wout_z[h] [128, D]: W_out rows for head h in rows 0..63 (+ bias row 64
      for h2), zeros elsewhere.

wqkv host layout [768, 704]: [q01(128) | k01(128) | q2k2(128) | k2q2(128)
| v012(192)] — the duplicated q2/k2 columns let both h2 operands land in
rows 0..63 without a cross-partition move.
"""

import numpy as np

import concourse.bass as bass
import concourse.tile as tile
from concourse import mybir, bacc
from concourse.bass_utils import run_bass_kernel_spmd
from concourse.masks import make_identity

F32 = mybir.dt.float32
F32R = mybir.dt.float32r
F16 = mybir.dt.float16
EXP = mybir.ActivationFunctionType.Exp

N_CORES = 8
B = 2
N = 4096
D = 768
H = 12
HD = 64  # head dim
SCALE = HD ** -0.5
DC = D // 128       # 6 contraction chunks
TC = N // 128       # 32 token chunks
QC = 1024           # query block
NQC = N // QC       # 4
NKC = N // 128      # 32 key chunks

TRACE = False
TRACE_ALL_CORES = False
LAST_RESULT = None

_nc_cache = None


def _build_module():
    nc = bacc.Bacc("TRN2", target_bir_lowering=False, debug=False,
                   num_devices=N_CORES)
    x_d = nc.dram_tensor("x", [N, D], F32, kind="ExternalInput")
    wqkv_d = nc.dram_tensor("wqkv", [D, 704], F32, kind="ExternalInput")
    wout_d = nc.dram_tensor("wout", [193, D], F32, kind="ExternalInput")
    y_d = nc.dram_tensor("y", [N, D], F32, kind="ExternalOutput")

    with tile.TileContext(nc) as tc:
        _emit(nc, tc, x_d, wqkv_d, wout_d, y_d)
    nc.compile()
    return nc


def _emit(nc, tc, x_d, wqkv_d, wout_d, y_d):
    from contextlib import ExitStack
    ctx = ExitStack()
    with ctx:
        consts = ctx.enter_context(tc.tile_pool(name="consts", bufs=1))
        weights = ctx.enter_context(tc.tile_pool(name="weights", bufs=1))
        qkvp = ctx.enter_context(tc.tile_pool(name="qkv", bufs=1))
        apool = ctx.enter_context(tc.tile_pool(name="attnout", bufs=1))

        # --- constants ---------------------------------------------------
        ident32 = consts.tile([128, 128], F32, tag="id32")
        make_identity(nc, ident32[:])
        ident = consts.tile([128, 128], F16, tag="id")
        nc.vector.tensor_copy(ident[:], ident32[:])
        ones_kc = consts.tile([128, NKC, 1], F32, tag="ones_kc")
        nc.vector.memset(ones_kc[:], 1.0)
        ones_q = consts.tile([1, QC], F32, tag="ones_q")
        nc.vector.memset(ones_q[:], 1.0)

        # --- weights / persistent activations ---------------------------
        wqkv = weights.tile([128, DC, 704], F16, tag="wqkv")
        wout_z = [weights.tile([128, D], F16, tag=f"woz{h}", name=f"woz{h}")
                  for h in range(3)]
        qTz = [qkvp.tile([128, N], F16, tag=f"qTz{h}", name=f"qTz{h}")
               for h in range(3)]
        kTz = [qkvp.tile([128, N], F16, tag=f"kTz{h}", name=f"kTz{h}")
               for h in range(3)]
        v128 = [qkvp.tile([128, NKC, 128], F16, tag=f"v{h}", name=f"v{h}")
                for h in range(3)]
        A_z = [apool.tile([128, N], F16, tag=f"Az{h}", name=f"Az{h}")
               for h in range(3)]

        # zero-fill the padded halves (one-time)
        for h in range(3):
            nc.vector.memset(qTz[h][:], 0.0)
            nc.vector.memset(kTz[h][:], 0.0)
            nc.vector.memset(v128[h][:], 0.0)
            nc.vector.memset(A_z[h][:], 0.0)
            nc.vector.memset(wout_z[h][:], 0.0)
            nc.vector.tensor_copy(v128[h][:, :, 64:65], ones_kc[:])
        for qb in range(NQC):
            nc.vector.tensor_copy(A_z[2][64:65, qb * QC:(qb + 1) * QC], ones_q[:])

        # ================= phase A: transpose + projections ==============
        with tc.tile_pool(name="xT", bufs=1) as xTp, \
             tc.tile_pool(name="xcp", bufs=2) as xcp, \
             tc.tile_pool(name="w32p", bufs=1) as w32p, \
             tc.tile_pool(name="tps", bufs=2, space=bass.MemorySpace.PSUM) as tps, \
             tc.tile_pool(name="qkps", bufs=2, space=bass.MemorySpace.PSUM) as qkps, \
             tc.tile_pool(name="vps", bufs=2, space=bass.MemorySpace.PSUM) as vps:
            wqkv32 = w32p.tile([128, DC, 704], F32, tag="wqkv32")
            nc.sync.dma_start(
                wqkv32[:], wqkv_d.ap().rearrange("(c p) m -> p c m", p=128))
            nc.vector.tensor_copy(wqkv[:], wqkv32[:])
            wo32 = w32p.tile([128, 2, D], F32, tag="wo32")
            nc.sync.dma_start(wo32[0:64, 0, :], wout_d.ap()[0:64, :])
            nc.sync.dma_start(wo32[64:128, 0, :], wout_d.ap()[64:128, :])
            nc.sync.dma_start(wo32[0:65, 1, :], wout_d.ap()[128:193, :])
            nc.vector.tensor_copy(wout_z[0][0:64, :], wo32[0:64, 0, :])
            nc.vector.tensor_copy(wout_z[1][64:128, :], wo32[64:128, 0, :])
            nc.vector.tensor_copy(wout_z[2][0:65, :], wo32[0:65, 1, :])

            NSEG = 4
            SEG = N // NSEG
            SEGC = SEG // 128
            for seg in range(NSEG):
                t0 = seg * SEGC
                col0 = seg * SEG
                xT = xTp.tile([128, DC, SEG], F16, tag="xT")
                for tg in range(SEGC // 4):
                    xc32 = xcp.tile([128, 4, D], F32, tag="xc32")
                    nc.sync.dma_start(
                        xc32[:],
                        x_d.ap()[(t0 + tg * 4) * 128:(t0 + tg * 4 + 4) * 128, :]
                        .rearrange("(g p) d -> p g d", p=128))
                    xc = xcp.tile([128, 4, D], F16, tag="xc")
                    nc.vector.tensor_copy(xc[:], xc32[:])
                    for ti in range(4):
                        t = tg * 4 + ti
                        tp = tps.tile([128, DC, 128], F16, tag="tp")
                        for c in range(DC):
                            nc.tensor.transpose(tp[:, c, :],
                                                xc[:, ti, c * 128:(c + 1) * 128],
                                                ident[:])
                        nc.vector.tensor_copy(xT[:, :, t * 128:(t + 1) * 128], tp[:])
                # fused q/k projections; chunk -> (dst tile, dst row half)
                for ci, copies in ((0, ((qTz[0], 0), (qTz[1], 1))),
                                   (1, ((kTz[0], 0), (kTz[1], 1))),
                                   (2, ((qTz[2], 0),)),
                                   (3, ((kTz[2], 0),))):
                    c0 = 128 * ci
                    for nb in range(SEG // 512):
                        acc = qkps.tile([128, 512], F32, tag="qkps")
                        for c in range(DC):
                            nc.tensor.matmul(acc[:], wqkv[:, c, c0:c0 + 128],
                                             xT[:, c, nb * 512:(nb + 1) * 512],
                                             start=(c == 0), stop=(c == DC - 1))
                        cc = col0 + nb * 512
                        for dst, half in copies:
                            lo, hi = (0, 64) if half == 0 else (64, 128)
                            nc.vector.tensor_copy(dst[lo:hi, cc:cc + 512],
                                                  acc[lo:hi, :])
                # v projection (normal orientation, xT chunks as stationary)
                for t in range(SEGC):
                    acc = vps.tile([128, 192], F32, tag="vps")
                    for c in range(DC):
                        nc.tensor.matmul(acc[:], xT[:, c, t * 128:(t + 1) * 128],
                                         wqkv[:, c, 512:704],
                                         start=(c == 0), stop=(c == DC - 1))
                    for h in range(3):
                        nc.vector.tensor_copy(v128[h][:, t0 + t, 0:64],
                                              acc[:, 64 * h:64 * h + 64])

        # ========= phase B: flash attention + fused output projection ====
        with tc.tile_pool(name="sps", bufs=2, space=bass.MemorySpace.PSUM) as sps, \
             tc.tile_pool(name="ops", bufs=2, space=bass.MemorySpace.PSUM) as ops, \
             tc.tile_pool(name="pp", bufs=3) as pp, \
             tc.tile_pool(name="osbp", bufs=2) as osbp, \
             tc.tile_pool(name="ysbp", bufs=3) as ysbp, \
             tc.tile_pool(name="rp", bufs=2) as rp, \
             tc.tile_pool(name="rbp", bufs=2) as rbp:
            for qb in range(NQC):
                q0 = qb * QC
                for h in range(3):
                    o = ops.tile([128, QC], F32, tag="o")
                    for kc in range(NKC):
                        s = sps.tile([128, QC], F32, tag="s")
                        for j in (0, 512):
                            nc.tensor.matmul(s[:, j:j + 512],
                                             kTz[h][:, kc * 128:(kc + 1) * 128],
                                             qTz[h][:, q0 + j:q0 + j + 512],
                                             start=True, stop=True)
                        p = pp.tile([128, QC], F16, tag="p")
                        nc.scalar.activation(p[:], s[:], EXP, scale=SCALE)
                        for j in (0, 512):
                            nc.tensor.matmul(o[:, j:j + 512], v128[h][:, kc, :],
                                             p[:, j:j + 512],
                                             start=(kc == 0), stop=(kc == NKC - 1))
                    osb = osbp.tile([65, QC], F32, tag="osb")
                    nc.vector.tensor_copy(osb[:], o[0:65, :])
                    rc = rp.tile([1, QC], F32, tag="rc")
                    nc.vector.reciprocal(rc[:], osb[64:65, :])
                    rcb = rbp.tile([64, QC], F32, tag="rcb")
                    nc.gpsimd.partition_broadcast(rcb[:], rc[:])
                    nc.vector.tensor_mul(A_z[h][0:64, q0:q0 + QC],
                                         osb[0:64, :], rcb[:])
                # output projection for this query block (PSUM slots shared
                # with the attention accumulators via the same pool tag)
                for t in range(qb * (QC // 128), (qb + 1) * (QC // 128)):
                    ts = slice(t * 128, (t + 1) * 128)
                    y = ops.tile([128, D], F32, tag="o", name="y")
                    for c0, c1 in ((0, 512), (512, 768)):
                        for h in range(3):
                            nc.tensor.matmul(y[:, c0:c1], A_z[h][:, ts],
                                             wout_z[h][:, c0:c1],
                                             start=(h == 0), stop=(h == 2))
                    ysb = ysbp.tile([128, D], F32, tag="ysb")
                    nc.vector.tensor_copy(ysb[:], y[:])
                    nc.sync.dma_start(y_d.ap()[ts, :], ysb[:])


def _get_nc():
    global _nc_cache
    if _nc_cache is None:
        _nc_cache = _build_module()
    return _nc_cache


def kernel(x, W_qkv, W_out, b_out):
    global LAST_RESULT
    x = np.asarray(x, dtype=np.float32)
    W_qkv = np.asarray(W_qkv, dtype=np.float32)
    W_out = np.asarray(W_out, dtype=np.float32)
    b_out = np.asarray(b_out, dtype=np.float32)

    in_maps = []
    for c in range(N_CORES):
        b, j = divmod(c, 4)
        h0 = 3 * j
        q0, k0, v0 = 64 * h0, D + 64 * h0, 2 * D + 64 * h0
        q01 = W_qkv[:, q0:q0 + 128]
        k01 = W_qkv[:, k0:k0 + 128]
        q2 = W_qkv[:, q0 + 128:q0 + 192]
        k2 = W_qkv[:, k0 + 128:k0 + 192]
        v012 = W_qkv[:, v0:v0 + 192]
        wqkv_slice = np.ascontiguousarray(
            np.concatenate([q01, k01, q2, k2, k2, q2, v012], axis=1))
        r0 = 64 * h0
        bias_row = b_out[None, :] if j == 0 else np.zeros((1, D), np.float32)
        wout_slice = np.ascontiguousarray(np.concatenate(
            [W_out[r0:r0 + 192], bias_row], axis=0))
        in_maps.append({
            "x": np.ascontiguousarray(x[b]),
            "wqkv": wqkv_slice,
            "wout": wout_slice,
        })

    nc = _get_nc()
    kwargs = {}
    if TRACE:
        from concourse import bass_utils as _bu
        _bu.upload_artifacts = lambda tmpdir: "local://" + tmpdir
        kwargs["trace"] = True
        if TRACE_ALL_CORES:
            kwargs["trace_cores"] = list(range(N_CORES))
    res = run_bass_kernel_spmd(nc, in_maps, core_ids=list(range(N_CORES)), **kwargs)
    LAST_RESULT = res

    out = np.empty((B, N, D), dtype=np.float32)
    for b in range(B):
        out[b] = (res.results[4 * b + 0]["y"] + res.results[4 * b + 1]["y"]
                  + res.results[4 * b + 2]["y"] + res.results[4 * b + 3]["y"])
    return out
